# revision 31
# baseline (speedup 1.0000x reference)
"""Trainium2 distributed kernel for nn_CPAM_Module (CPAM attention block).

Math collapse (verified exact vs reference, ~2.6e-8 fro rel err in f64):
  te   = text_flat @ G_w.T + G_b                      (B, C)
  te_flat = te[:, :, None] * l  (rank-1 per batch)  =>
  proj_key / proj_value are rank-1 in n; energy[b,n,m] = s[b,n]*l[m] + const(n)
  softmax over m kills the const =>
  attn[b,n,m] = softmax_m(s[b,n] * l[m])
  s[b,n] = sum_c u[b,c] x[b,c,n] + b_q.kte[b],  u = kte @ W_q, kte = te @ W_k.T
  a[b,n] = (sum_j l_j e^{l_j s}) / (sum_j e^{l_j s})
  out    = gamma * (vte[b,c] * a[b,n] + b_v[c]) + x,  vte = te @ W_v.T

Sharding: contraction (TXT=153600) split 8 ways for the big G matmul;
ReduceScatter of te (bf16) hands each core its 32 batches; epilogue is
batch-parallel. x/out traffic is B-sharded (25.7 MB each per core).

Structure:
- G matmul split into two C-halves with the text tiles resident in SBUF:
  half A's te is reduce-scattered, PE-transposed and prepped (kte/u/gvte
  partial accumulation) while half B's matmul still streams, so only half
  B's short tail is exposed and the PE never idles long enough to drop
  its p-state.
- s[b,n] = u.x computed on PE (8 accumulating matmuls/batch); softmax row
  via outer-product matmuls, one merged exp on ACT, den|num side by side
  on partition 0; out planes via per-pair gvte (x) a outer products + two
  paired adds on DVE and an ACT-evac + Pool add for the rest (GPSIMD
  cannot read PSUM on silicon).
- x/out DRAM layouts carry two batches per row so each DMA moves a pair
  (halves HWDGE config + completion-semaphore overhead); x pairs queue on
  SP *behind* the g_w stream and are then paced one per round from
  stage_a, so they fill the RS/prep gap without delaying te completion.
- 1/GW_SCALE and gamma folded into W_k/W_v host-side; per-batch work
  software-pipelined in 4 stages across rounds to hide the z->exp->nd
  and div->outer chains under the s-block matmuls.
"""

import sys

sys.path.insert(0, "/opt/trn_rl_repo")

import numpy as np
import ml_dtypes

from concourse import bass, bacc, mybir, tile
from concourse.bass_utils import run_bass_kernel_spmd

F32 = mybir.dt.float32
BF16 = mybir.dt.bfloat16
FP8 = mybir.dt.float8e4
GW_SCALE = 256.0
AF = mybir.ActivationFunctionType
ALU = mybir.AluOpType

N_CORES = 8
B, C, H, W = 256, 1024, 14, 14
N = H * W  # 196
C8 = 128
TXT = 150 * 1024
KSH = TXT // N_CORES  # 19200 txt-contraction shard per core
NK = KSH // 128  # 150 k-tiles
BL = B // N_CORES  # 32 local batches
CT = C // 128  # 8 c tiles
JT = 98  # j-tile (196 = 2*98)


FCLAMP = 120.0  # f(s) polynomial fit/clamp range


def build(gamma: float, skip_gb: bool, skip_bq: bool, skip_bv: bool, single: bool = False, repeat: int = 1, loop_n: int = 0, part: str = 'all', fcoef=(0.0,) * 4):
    # single=True builds a 1-core variant with the ReduceScatter replaced by a
    # local DMA (same bytes landing in te_rs) so TimelineSim can model it.
    nc = bacc.Bacc(
        "TRN2",
        target_bir_lowering=False,
        debug=False,
        num_devices=1 if single else N_CORES,
    )

    text_t = nc.dram_tensor("text_t", [128, NK * B], FP8, kind="ExternalInput")
    g_wt = nc.dram_tensor("g_wt", [128, NK * C], FP8, kind="ExternalInput")
    # x and out carry two batches per row so each DMA moves a pair (fewer
    # HWDGE configs + completion semaphores)
    xs = nc.dram_tensor("xs", [BL // 2, 128, 2 * CT * N], BF16, kind="ExternalInput")
    w_vt = nc.dram_tensor("w_vt", [128, CT * C], BF16, kind="ExternalInput")
    w_kt = nc.dram_tensor("w_kt", [128, CT * C8], BF16, kind="ExternalInput")
    w_q = nc.dram_tensor("w_q", [C8, C], BF16, kind="ExternalInput")
    id32 = nc.dram_tensor("id32", [BL, BL], BF16, kind="ExternalInput")
    idf = nc.dram_tensor("idf", [128, 128], BF16, kind="ExternalInput")
    g_b = nc.dram_tensor("g_b", [C8, CT], F32, kind="ExternalInput")
    b_q = nc.dram_tensor("b_q", [C8, 1], BF16, kind="ExternalInput")
    gbv = nc.dram_tensor("gbv", [C8, CT], F32, kind="ExternalInput")
    out = nc.dram_tensor("out", [BL // 2, 128, 2 * CT * N], BF16, kind="ExternalOutput")

    with tile.TileContext(nc) as tc:
        with (
            tc.tile_pool(name="const", bufs=1) as const,
            tc.tile_pool(name="dram", bufs=1, space="DRAM") as dram,
        ):
            # Constants
            id_sb = const.tile([BL, BL], BF16, tag="id32")
            nc.sync.dma_start(id_sb[:], id32[:, :])
            idf_sb = const.tile([128, 128], BF16, tag="idf")
            nc.sync.dma_start(idf_sb[:], idf[:, :])
            wvt_sb = const.tile([128, CT, C], BF16, tag="wvt")
            nc.scalar.dma_start(wvt_sb[:].opt(), w_vt[:, :])
            wkt_sb = const.tile([128, CT, C8], BF16, tag="wkt")
            nc.scalar.dma_start(wkt_sb[:].opt(), w_kt[:, :])
            wq_sb = const.tile([C8, C], BF16, tag="wq")
            nc.sync.dma_start(wq_sb[:], w_q[:, :])
            if not skip_gb:
                gb_sb = const.tile([C8, CT], F32, tag="gb")
                nc.sync.dma_start(gb_sb[:], g_b[:, :])
            if not skip_bq:
                bq_sb = const.tile([C8, 1], BF16, tag="bq")
                nc.sync.dma_start(bq_sb[:], b_q[:, :])
            if not skip_bv:
                gbv_sb = const.tile([C8, CT], F32, tag="gbv")
                nc.sync.dma_start(gbv_sb[:], gbv[:, :])
            te_f = [dram.tile([B, 512], BF16, name=f"te_f{h}") for h in range(2)]
            te_r = [dram.tile([BL, 512], BF16, name=f"te_r{h}") for h in range(2)]

            if loop_n:
                assert single, "hardware loop timing mode is single-core only"
                loop_cm = tc.For_i(0, loop_n, 1)
                loop_cm.__enter__()
            for _rep in range(repeat):
                with (
                    tc.tile_pool(name=f"xp{_rep}", bufs=12) as xp,
                    tc.tile_pool(name=f"small{_rep}", bufs=2) as sm,
                    tc.tile_pool(name=f"a4p{_rep}", bufs=3) as a4p,
                    tc.tile_pool(name=f"gr{_rep}", bufs=3) as grp,
                    tc.tile_pool(name=f"op{_rep}", bufs=3) as op,
                ):
                    xtiles = {}
                    tls = []

                    def xload(b2, eng=None):
                        # Loads the batch pair (2*b2, 2*b2+1). On the SP queue:
                        # FIFO order keeps these *behind* the g_w stream so they
                        # don't delay te-completion, then they fill the RS/prep
                        # gap and feed the epilogue. The first few go on the
                        # ACT queue (immediate) so the leading epilogue rounds
                        # aren't DMA-gated.
                        xb = xp.tile([128, 2, CT, N], BF16, tag="xb", name=f"xb{_rep}_{b2}")
                        (eng or nc.sync).dma_start(xb[:].opt(), xs[b2].opt())
                        xtiles[b2] = xb

                    # ---- Phases 1-3: G matmul in two C-halves + ReduceScatter +
                    # prep. Half A's te is reduced/transposed/prepped while half
                    # B's matmul still streams, so only half B's short tail is
                    # exposed and the PE never idles long enough to lose p-state.
                    teT_sb = const.tile([128, CT, BL], BF16, tag="teT", name=f"teT{_rep}")
                    # uT with wrapped duplicate columns so the M=32 col-packed
                    # s matmuls can slice [b : b+32] for any b
                    uT2_sb = const.tile([128, CT, 2 * BL], BF16, tag="uT2", name=f"uT2{_rep}")
                    gvr_sb = const.tile([BL, C], BF16, tag="gvr", name=f"gvr{_rep}")
                    bqd_row = const.tile([1, BL], F32, tag="bqd", name=f"bqd{_rep}") if not skip_bq else None
                    NPRE = 10  # x pairs loaded before the epilogue starts
                    te_sbh = [None, None]
                    gv_ps = [None, None]
                    kteT_ps = None

                    KB = 10  # k-tiles per DMA batch (150 = 15 * 10)
                    NPAIR = NK // 2
                    NG = NK // KB

                    with (
                        tc.tile_pool(name=f"gpsum{_rep}", bufs=2, space="PSUM") as gp,
                        tc.tile_pool(name=f"tl{_rep}", bufs=NG) as tlp,
                        tc.tile_pool(name=f"gw{_rep}", bufs=3) as gwp,
                        tc.tile_pool(name=f"tesb{_rep}", bufs=4) as tesb,
                        tc.tile_pool(name=f"pst{_rep}", bufs=2, space="PSUM") as ppst,
                        tc.tile_pool(name=f"pkte{_rep}", bufs=1, space="PSUM") as ppk,
                        tc.tile_pool(name=f"pgv{_rep}", bufs=1, space="PSUM") as ppg,
                        tc.tile_pool(name=f"pups{_rep}", bufs=1, space="PSUM") as ppu,
                        tc.tile_pool(name=f"psmall{_rep}", bufs=2) as psm,
                    ):

                        def rs_half(h, pth):
                            # evacuate the half's psums and reduce-scatter
                            for m in range(2):
                                ev = tesb.tile([128, 512], BF16, tag="tesb")
                                if m == 0:
                                    nc.vector.tensor_copy(ev[:], pth[m][:])
                                else:
                                    nc.scalar.copy(ev[:], pth[m][:])
                                nc.sync.dma_start(
                                    te_f[h][m * 128 : (m + 1) * 128, :], ev[:]
                                )
                            if single:
                                nc.sync.dma_start(te_r[h][:, :], te_f[h][0:BL, :])
                            else:
                                nc.gpsimd.collective_compute(
                                    "ReduceScatter",
                                    ALU.add,
                                    replica_groups=[list(range(N_CORES))],
                                    ins=[te_f[h].opt()],
                                    outs=[te_r[h].opt()],
                                )
                            te_sbh[h] = const.tile(
                                [BL, 512], BF16, tag="te_sb", name=f"te_sb{_rep}_{h}"
                            )
                            nc.scalar.dma_start(te_sbh[h][:], te_r[h][:, :])

                        def prep_half(h):
                            # transposes + partial kteT / gvte accumulation for
                            # the half's 4 c-tiles
                            for tt in range(4):
                                t = h * 4 + tt
                                pst = ppst.tile([128, BL], BF16, tag="pst")
                                nc.tensor.transpose(
                                    pst[:], te_sbh[h][:, tt * 128 : (tt + 1) * 128], id_sb[:]
                                )
                                if tt % 2 == 0:
                                    nc.vector.tensor_copy(teT_sb[:, t, :], pst[:])
                                else:
                                    nc.scalar.copy(teT_sb[:, t, :], pst[:])
                                if not skip_gb:
                                    nc.vector.tensor_scalar_add(
                                        teT_sb[:, t, :], teT_sb[:, t, :], gb_sb[:, t : t + 1]
                                    )
                            for tt in range(4):
                                t = h * 4 + tt
                                nc.tensor.matmul(
                                    kteT_ps[:],
                                    wkt_sb[:, t, :],
                                    teT_sb[:, t, :],
                                    start=(t == 0),
                                    stop=(t == CT - 1),
                                )
                            for h2 in range(2):
                                for tt in range(4):
                                    t = h * 4 + tt
                                    nc.tensor.matmul(
                                        gv_ps[h2][:],
                                        teT_sb[:, t, :],
                                        wvt_sb[:, t, h2 * 512 : (h2 + 1) * 512],
                                        start=(t == 0),
                                        stop=(t == CT - 1),
                                    )

                        def prep_tail():
                            # kteT/gvte evacs, uT, bqd -- after both halves landed
                            kteT_sb = psm.tile([C8, BL], BF16, tag="kteT")
                            nc.vector.tensor_copy(kteT_sb[:], kteT_ps[:])
                            for h2 in range(2):
                                if h2 == 0:
                                    nc.vector.tensor_copy(
                                        gvr_sb[:, h2 * 512 : (h2 + 1) * 512], gv_ps[h2][:]
                                    )
                                else:
                                    nc.scalar.copy(
                                        gvr_sb[:, h2 * 512 : (h2 + 1) * 512], gv_ps[h2][:]
                                    )
                            for t in range(CT):
                                u_ps = ppu.tile([128, BL], F32, tag="ups")
                                nc.tensor.matmul(
                                    u_ps[:],
                                    wq_sb[:, t * 128 : (t + 1) * 128],
                                    kteT_sb[:],
                                    start=True,
                                    stop=True,
                                )
                                if t % 2 == 0:
                                    nc.vector.tensor_copy(uT2_sb[:, t, 0:BL], u_ps[:])
                                else:
                                    nc.scalar.copy(uT2_sb[:, t, 0:BL], u_ps[:])
                            nc.vector.tensor_copy(
                                uT2_sb[:, :, BL : 2 * BL], uT2_sb[:, :, 0:BL]
                            )
                            if not skip_bq:
                                bq_ps = ppu.tile([BL, 1], F32, tag="bqps")
                                nc.tensor.matmul(bq_ps[:], kteT_sb[:], bq_sb[:], start=True, stop=True)
                                bqd_col = psm.tile([BL, 1], F32, tag="bqdc")
                                nc.vector.tensor_copy(bqd_col[:], bq_ps[:])
                                nc.sync.dma_start(bqd_row[:].rearrange("o b -> o b 1"), bqd_col[:])

                        if part in ("all", "epi"):
                            kteT_ps = ppk.tile([C8, BL], F32, tag="kte")
                            for h2 in range(2):
                                gv_ps[h2] = ppg.tile(
                                    [BL, 512], F32, tag=f"gv{h2}", name=f"gv{_rep}_{h2}"
                                )

                        def g_pass(h):
                            pth = [
                                gp.tile([128, 512], F32, tag="gp", name=f"gp{_rep}_{h}{m}")
                                for m in range(2)
                            ]
                            for g in range(NG):
                                if h == 0:
                                    tl = tlp.tile(
                                        [128, KB, B], FP8, tag="tl", name=f"tl{_rep}_{g}"
                                    )
                                    tls.append(tl)
                                    nc.sync.dma_start(
                                        tl[:].opt(), text_t[:, g * KB * B : (g + 1) * KB * B]
                                    )
                                gw_t = gwp.tile([128, KB, 512], FP8, tag="gw")
                                off = (h * NK + g * KB) * 512
                                nc.sync.dma_start(
                                    gw_t[:].opt(), g_wt[:, off : off + KB * 512]
                                )
                                for f in range(0, KB, 2):
                                    j = (g * KB + f) // 2  # pair index
                                    for m in range(2):
                                        nc.tensor.matmul(
                                            pth[m][:],
                                            tls[g][:, f : f + 2, m * 128 : (m + 1) * 128],
                                            gw_t[:, f : f + 2, :],
                                            start=(j == 0),
                                            stop=(j == NPAIR - 1),
                                            perf_mode=mybir.MatmulPerfMode.DoubleRow,
                                        )
                                # interleave half A's prep into half B's
                                # matmul stream (te_sbh[0] has landed by then)
                                if h == 1 and part == "all" and g == 8:
                                    prep_half(0)
                            return pth

                        if part in ("all", "g"):
                            for h in range(2):
                                pth = g_pass(h)
                                rs_half(h, pth)
                            if part == "all":
                                # NPRE pairs now; the rest paced from stage_a so
                                # their configs don't bury te_sb/gr on the DGE
                                for b2 in range(NPRE):
                                    xload(b2)

                        if part in ("all", "epi"):
                            if part == "epi":
                                for b2 in range(BL // 2):
                                    xload(b2)
                                for h in range(2):
                                    te_sbh[h] = const.tile(
                                        [BL, 512], BF16, tag="te_sb", name=f"te_sb{_rep}_{h}"
                                    )
                                    nc.scalar.dma_start(te_sbh[h][:], te_r[h][:, :])
                                prep_half(0)
                            prep_half(1)
                            prep_tail()

                    # ---- Phase 4: round-based epilogue, 4 batches (2 x-pairs)
                    # per round. s for a batch PAIR comes from ONE M=32 N=392
                    # matmul per c-tile: rhs = [x_b0_t | x_b1_t], lhsT = the
                    # wrapped uT2 window, so b0's s lands at row 64p cols 0:196
                    # and b1's at row 64p+1 cols 196:392 (other rows garbage
                    # but initialized). The softmax block collapses to
                    # a[n] = f(s[n]) (f = softmax-expectation of the fixed l),
                    # evaluated as a host-fitted degree-3 polynomial in 6 bf16
                    # DVE ops on the full [128,392] tile. The good a blocks are
                    # DMA-hopped into a zeroed block-diagonal tile, so each
                    # outer product covers a batch pair per c-tile via one K=2
                    # N=392 matmul. +x via DVE-TT / PE id-add / Pool (MODES).
                    NR = BL // 4  # 8 rounds
                    # per-(c-group, pair) evac mode knobs (8 units/round)
                    MODES = [
                        ("dve", "pe"),
                        ("pool", "dve"),
                        ("dve", "pool"),
                        ("pe", "dve"),
                    ]
                    c3, c2, c1, c0 = fcoef
                    adiags = []
                    for _d in range(2):
                        ad = const.tile([66, 2 * N], BF16, tag=f"adiag{_d}", name=f"adiag{_rep}_{_d}")
                        nc.gpsimd.memset(ad[:], 0.0)
                        adiags.append(ad)
                    with (
                        tc.tile_pool(name=f"ps_s4{_rep}", bufs=2, space="PSUM") as ps_s4,
                        tc.tile_pool(name=f"ps_pr{_rep}", bufs=3, space="PSUM") as ps_pr,
                    ):
                        rs = {}

                        def stage_a(r):
                            if part == "all" and NPRE + r < BL // 2:
                                xload(NPRE + r)
                            st = {}
                            # gvte pair rows hop to bases 0 / 64 (outer lhsT)
                            gr2 = grp.tile([66, C], BF16, tag="gr2", name=f"gr2_{_rep}_{r}")
                            for p in range(2):
                                b = 4 * r + 2 * p
                                nc.scalar.dma_start(
                                    gr2[64 * p : 64 * p + 2, :], gvr_sb[b : b + 2, :]
                                )
                            # s block: one MM per (pair, c-tile); pair p's two
                            # batches land at rows (64p, 64p+1) x col halves
                            s4 = ps_s4.tile([128, 512], F32, tag="s4")
                            for p in range(2):
                                b = 4 * r + 2 * p
                                xb = xtiles[b // 2]
                                for t in range(CT):
                                    nc.tensor.matmul(
                                        s4[64 * p : 64 * p + 32, 0 : 2 * N],
                                        uT2_sb[:, t, b : b + 32],
                                        xb[:, :, t, :],
                                        start=(t == 0),
                                        stop=(t == CT - 1),
                                        tile_position=(0, 64 * p),
                                    )
                            st["s"] = s4
                            st["gr"] = gr2
                            rs[r] = st

                        def stage_b(r):
                            # a = f(s): clamp (psum read) then degree-3 poly
                            # as odd/even parts in sigma^2, all bf16 on DVE
                            st = rs[r]
                            s4 = st["s"]
                            TS = nc.vector.tensor_scalar
                            TT = nc.vector.tensor_tensor
                            sg = sm.tile([128, 2 * N], BF16, tag="sg")
                            TS(sg[:], s4[:, 0 : 2 * N], FCLAMP, -FCLAMP, ALU.min, ALU.max)
                            if not skip_bq:
                                bq4 = sm.tile([128, 1], F32, tag="bq4")
                                for p in range(2):
                                    for g in range(2):
                                        b = 4 * r + 2 * p + g
                                        nc.scalar.dma_start(
                                            bq4[64 * p + g : 64 * p + g + 1, :],
                                            bqd_row[0:1, b : b + 1],
                                        )
                                for p in range(2):
                                    for g in range(2):
                                        row = 64 * p + g
                                        nc.vector.tensor_scalar_add(
                                            sg[row : row + 1, g * N : (g + 1) * N],
                                            sg[row : row + 1, g * N : (g + 1) * N],
                                            bq4[row : row + 1, :],
                                        )
                            s2 = sm.tile([128, 2 * N], BF16, tag="s2")
                            TT(s2[:], sg[:], sg[:], ALU.mult)
                            ao = sm.tile([128, 2 * N], BF16, tag="ao")
                            TS(ao[:], s2[:], c3, c1, ALU.mult, ALU.add)
                            be = sm.tile([128, 2 * N], BF16, tag="be")
                            TS(be[:], s2[:], c2, c0, ALU.mult, ALU.add)
                            TT(ao[:], ao[:], sg[:], ALU.mult)
                            a4 = a4p.tile([128, 2 * N], BF16, tag="a4", name=f"a4_{_rep}_{r}")
                            TT(a4[:], ao[:], be[:], ALU.add)
                            # good blocks hop into the zeroed block-diag tile
                            adiag = adiags[r % 2]
                            for p in range(2):
                                for g in range(2):
                                    row = 64 * p + g
                                    nc.scalar.dma_start(
                                        adiag[row : row + 1, g * N : (g + 1) * N],
                                        a4[row : row + 1, g * N : (g + 1) * N],
                                    )
                            st["a"] = adiag

                        def stage_c(r):
                            # K=2 outers: one MM per (pair, c-tile) covers both
                            # batches; +x via DVE-TT / PE id-add / Pool (MODES)
                            st = rs.pop(r)
                            gr2 = st["gr"]
                            adiag = st["a"]
                            obs = [
                                op.tile([128, 2, CT, N], BF16, tag="ob", name=f"ob{_rep}_{2 * r + i}")
                                for i in range(2)
                            ]
                            for p in range(2):
                                xb = xtiles[2 * r + p]
                                ob = obs[p]
                                for cg in range(4):
                                    mode = MODES[cg][p]
                                    pr = ps_pr.tile([128, 2, 512], F32, tag="pr")
                                    for h in range(2):
                                        t = 2 * cg + h
                                        nc.tensor.matmul(
                                            pr[:, h, 0 : 2 * N],
                                            gr2[64 * p : 64 * p + 2, t * 128 : (t + 1) * 128],
                                            adiag[64 * p : 64 * p + 2, :],
                                            start=True,
                                            stop=(mode != "pe"),
                                            tile_position=(64 * p, 0),
                                        )
                                    pr_ap = pr[:, :, 0 : 2 * N].rearrange(
                                        "c t (g n) -> c g t n", g=2
                                    )
                                    ob_ap = ob[:, :, 2 * cg : 2 * cg + 2, :]
                                    xb_ap = xb[:, :, 2 * cg : 2 * cg + 2, :]
                                    if mode == "pe":
                                        for h in range(2):
                                            t = 2 * cg + h
                                            nc.tensor.matmul(
                                                pr[:, h, 0 : 2 * N],
                                                idf_sb[:, :],
                                                xb[:, :, t, :],
                                                start=False,
                                                stop=True,
                                                skip_group_check=True,
                                            )
                                        nc.scalar.copy(ob_ap, pr_ap)
                                    elif mode == "dve":
                                        nc.vector.tensor_tensor(
                                            ob_ap, pr_ap, xb_ap, ALU.add
                                        )
                                    else:  # pool
                                        tmp = sm.tile([128, 2, 2, N], BF16, tag="tmp")
                                        nc.scalar.copy(tmp[:], pr_ap)
                                        nc.gpsimd.tensor_add(ob_ap, tmp[:], xb_ap)
                                if not skip_bv:
                                    for g in range(2):
                                        for t in range(CT):
                                            nc.vector.tensor_scalar_add(
                                                ob[:, g, t, :],
                                                ob[:, g, t, :],
                                                gbv_sb[:, t : t + 1],
                                            )
                                nc.sync.dma_start(out[2 * r + p].opt(), ob[:].opt())

                        for rr in range(NR + 2 if part in ("all", "epi") else 0):
                            if 1 <= rr <= NR:
                                stage_b(rr - 1)
                            if rr < NR:
                                stage_a(rr)
                            if rr >= 2:
                                stage_c(rr - 2)
            if loop_n:
                loop_cm.__exit__(None, None, None)

    nc.compile()
    return nc


def _prep_inputs(inputs):
    """Host-side sharding. Returns in_maps for the 8 cores."""
    x = np.ascontiguousarray(inputs["x"], dtype=np.float32).reshape(B, C, N)
    text = np.ascontiguousarray(inputs["text_embed"], dtype=np.float32).reshape(B, -1)
    G_w = np.asarray(inputs["G_w"], dtype=np.float32)
    l = np.asarray(inputs["l"], dtype=np.float32).reshape(1, N)
    W_q = np.asarray(inputs["W_q"], dtype=np.float32)
    W_k = np.asarray(inputs["W_k"], dtype=np.float32)
    W_v = np.asarray(inputs["W_v"], dtype=np.float32)
    b_v = np.asarray(inputs["b_v"], dtype=np.float32)
    b_q = np.asarray(inputs["b_q"], dtype=np.float32)
    G_b = np.asarray(inputs["G_b"], dtype=np.float32)
    gamma = float(np.asarray(inputs["gamma"]).reshape(-1)[0])

    bf = ml_dtypes.bfloat16
    f8 = ml_dtypes.float8_e4m3

    def pretile(a, p=128):
        # (T*p, F) -> (p, T*F): partition-major tiling for contiguous DMA
        tp, f = a.shape
        t = tp // p
        return np.ascontiguousarray(a.reshape(t, p, f).transpose(1, 0, 2).reshape(p, t * f))

    # te is carried at GW_SCALE x through the ReduceScatter; fold the descale
    # (and gamma, for the value path) into the consumers of te.
    w_vt = pretile(np.ascontiguousarray(W_v.T * (gamma / GW_SCALE)).astype(bf))
    w_kt = pretile(np.ascontiguousarray(W_k.T / GW_SCALE).astype(bf))
    w_q = W_q.astype(bf)
    g_b_t = np.ascontiguousarray(G_b.reshape(CT, C8).T) * GW_SCALE  # (128, 8)
    gbv = np.ascontiguousarray((gamma * b_v).reshape(CT, C8).T)
    b_q_col = b_q.reshape(C8, 1).astype(bf)
    id32 = np.eye(BL, dtype=bf)
    idf = np.eye(128, dtype=bf)

    # Fit f(sigma) = sum_j l_j e^{l_j sigma} / sum_j e^{l_j sigma} (the
    # softmax-expectation of l -- a[n] = f(s[n]) pointwise) with a degree-3
    # Chebyshev polynomial over +-FCLAMP. Max fit error ~1.4e-3 of an output
    # delta that is ~1% of |x|, far inside the output tolerance.
    lv = l[0].astype(np.float64)

    def f_exact(sigma):
        z = np.outer(np.asarray(sigma, np.float64), lv)
        z -= z.max(axis=1, keepdims=True)
        e = np.exp(z)
        return (e @ lv) / e.sum(axis=1)

    gx = np.linspace(-FCLAMP, FCLAMP, 4001)
    cheb = np.polynomial.chebyshev.Chebyshev.fit(gx, f_exact(gx), 3)
    pc = cheb.convert(kind=np.polynomial.Polynomial).coef
    pc = np.pad(pc, (0, 4 - len(pc)))
    fcoef = tuple(float(c) for c in pc[::-1])  # (c3, c2, c1, c0)

    in_maps = []
    for i in range(N_CORES):
        sl = slice(i * KSH, (i + 1) * KSH)
        in_maps.append(
            {
                "text_t": pretile(np.ascontiguousarray(text[:, sl].T).astype(f8)),
                # C-halves-major so each G pass reads a contiguous half
                "g_wt": np.ascontiguousarray(
                    pretile((np.ascontiguousarray(G_w[:, sl].T) * GW_SCALE).astype(f8))
                    .reshape(128, NK, 2, 512)
                    .transpose(0, 2, 1, 3)
                    .reshape(128, NK * C)
                ),
                "xs": np.ascontiguousarray(
                    x[i * BL : (i + 1) * BL]
                    .reshape(BL // 2, 2, CT, 128, N)
                    .transpose(0, 3, 1, 2, 4)
                    .reshape(BL // 2, 128, 2 * CT * N)
                ).astype(bf),
                "w_vt": w_vt,
                "w_kt": w_kt,
                "w_q": w_q,
                "id32": id32,
                "idf": idf,
                "g_b": g_b_t,
                "b_q": b_q_col,
                "gbv": gbv,
            }
        )
    meta = {
        "gamma": gamma,
        "skip_gb": not np.any(G_b),
        "skip_bq": not np.any(b_q),
        "skip_bv": not np.any(b_v),
        "fcoef": fcoef,
    }
    return in_maps, meta


def _run(inputs, trace=False, repeat=1):
    in_maps, meta = _prep_inputs(inputs)
    nc = build(
        meta["gamma"], meta["skip_gb"], meta["skip_bq"], meta["skip_bv"],
        repeat=repeat, fcoef=meta["fcoef"],
    )
    res = run_bass_kernel_spmd(nc, in_maps, core_ids=list(range(N_CORES)), trace=trace)
    outs = [
        res.results[i]["out"]
        .astype(np.float32)
        .reshape(BL // 2, 128, 2, CT, N)
        .transpose(0, 2, 3, 1, 4)
        .reshape(BL, C, N)
        for i in range(N_CORES)
    ]
    full = np.concatenate(outs, axis=0).reshape(B, C, H, W)
    return full, res


def kernel(**inputs) -> np.ndarray:
    full, _ = _run(inputs, trace=False)
    return full


if __name__ == "__main__":
    import reference

    inputs = {k: np.asarray(v) for k, v in reference.setup_inputs().items()}
    got = kernel(**inputs)
    print("out shape:", got.shape, got.dtype)



# revision 32
# speedup vs baseline: 1.0700x; 1.0700x over previous
"""Trainium2 distributed kernel for nn_CPAM_Module (CPAM attention block).

Math collapse (verified exact vs reference, ~2.6e-8 fro rel err in f64):
  te   = text_flat @ G_w.T + G_b                      (B, C)
  te_flat = te[:, :, None] * l  (rank-1 per batch)  =>
  proj_key / proj_value are rank-1 in n; energy[b,n,m] = s[b,n]*l[m] + const(n)
  softmax over m kills the const =>
  attn[b,n,m] = softmax_m(s[b,n] * l[m])
  s[b,n] = sum_c u[b,c] x[b,c,n] + b_q.kte[b],  u = kte @ W_q, kte = te @ W_k.T
  a[b,n] = (sum_j l_j e^{l_j s}) / (sum_j e^{l_j s})
  out    = gamma * (vte[b,c] * a[b,n] + b_v[c]) + x,  vte = te @ W_v.T

Sharding: contraction (TXT=153600) split 8 ways for the big G matmul;
ReduceScatter of te (bf16) hands each core its 32 batches; epilogue is
batch-parallel. x/out traffic is B-sharded (25.7 MB each per core).

Structure:
- G matmul split into two C-halves with the text tiles resident in SBUF:
  half A's te is reduce-scattered, PE-transposed and prepped (kte/u/gvte
  partial accumulation) while half B's matmul still streams, so only half
  B's short tail is exposed and the PE never idles long enough to drop
  its p-state.
- s[b,n] = u.x computed on PE (8 accumulating matmuls/batch); softmax row
  via outer-product matmuls, one merged exp on ACT, den|num side by side
  on partition 0; out planes via per-pair gvte (x) a outer products + two
  paired adds on DVE and an ACT-evac + Pool add for the rest (GPSIMD
  cannot read PSUM on silicon).
- x/out DRAM layouts carry two batches per row so each DMA moves a pair
  (halves HWDGE config + completion-semaphore overhead); x pairs queue on
  SP *behind* the g_w stream and are then paced one per round from
  stage_a, so they fill the RS/prep gap without delaying te completion.
- 1/GW_SCALE and gamma folded into W_k/W_v host-side; per-batch work
  software-pipelined in 4 stages across rounds to hide the z->exp->nd
  and div->outer chains under the s-block matmuls.
"""

import sys

sys.path.insert(0, "/opt/trn_rl_repo")

import numpy as np
import ml_dtypes

from concourse import bass, bacc, mybir, tile
from concourse.bass_utils import run_bass_kernel_spmd

F32 = mybir.dt.float32
BF16 = mybir.dt.bfloat16
FP8 = mybir.dt.float8e4
GW_SCALE = 256.0
AF = mybir.ActivationFunctionType
ALU = mybir.AluOpType

N_CORES = 8
B, C, H, W = 256, 1024, 14, 14
N = H * W  # 196
C8 = 128
TXT = 150 * 1024
KSH = TXT // N_CORES  # 19200 txt-contraction shard per core
NK = KSH // 128  # 150 k-tiles
BL = B // N_CORES  # 32 local batches
CT = C // 128  # 8 c tiles
JT = 98  # j-tile (196 = 2*98)


FCLAMP = 120.0  # f(s) polynomial fit/clamp range


def build(gamma: float, skip_gb: bool, skip_bq: bool, skip_bv: bool, single: bool = False, repeat: int = 1, loop_n: int = 0, part: str = 'all', fcoef=(0.0,) * 4):
    # single=True builds a 1-core variant with the ReduceScatter replaced by a
    # local DMA (same bytes landing in te_rs) so TimelineSim can model it.
    nc = bacc.Bacc(
        "TRN2",
        target_bir_lowering=False,
        debug=False,
        num_devices=1 if single else N_CORES,
    )

    text_t = nc.dram_tensor("text_t", [128, NK * B], FP8, kind="ExternalInput")
    g_wt = nc.dram_tensor("g_wt", [128, NK * C], FP8, kind="ExternalInput")
    # x and out carry two batches per row so each DMA moves a pair (fewer
    # HWDGE configs + completion semaphores)
    xs = nc.dram_tensor("xs", [BL // 2, 128, 2 * CT * N], BF16, kind="ExternalInput")
    w_vt = nc.dram_tensor("w_vt", [128, CT * C], BF16, kind="ExternalInput")
    w_kt = nc.dram_tensor("w_kt", [128, CT * C8], BF16, kind="ExternalInput")
    w_q = nc.dram_tensor("w_q", [C8, C], BF16, kind="ExternalInput")
    id32 = nc.dram_tensor("id32", [BL, BL], BF16, kind="ExternalInput")
    idf = nc.dram_tensor("idf", [128, 128], BF16, kind="ExternalInput")
    g_b = nc.dram_tensor("g_b", [C8, CT], F32, kind="ExternalInput")
    b_q = nc.dram_tensor("b_q", [C8, 1], BF16, kind="ExternalInput")
    gbv = nc.dram_tensor("gbv", [C8, CT], F32, kind="ExternalInput")
    out = nc.dram_tensor("out", [BL // 2, 128, 2 * CT * N], BF16, kind="ExternalOutput")

    with tile.TileContext(nc) as tc:
        with (
            tc.tile_pool(name="const", bufs=1) as const,
            tc.tile_pool(name="dram", bufs=1, space="DRAM") as dram,
        ):
            # Constants
            id_sb = const.tile([BL, BL], BF16, tag="id32")
            nc.sync.dma_start(id_sb[:], id32[:, :])
            idf_sb = const.tile([128, 128], BF16, tag="idf")
            nc.sync.dma_start(idf_sb[:], idf[:, :])
            wvt_sb = const.tile([128, CT, C], BF16, tag="wvt")
            nc.scalar.dma_start(wvt_sb[:].opt(), w_vt[:, :])
            wkt_sb = const.tile([128, CT, C8], BF16, tag="wkt")
            nc.scalar.dma_start(wkt_sb[:].opt(), w_kt[:, :])
            wq_sb = const.tile([C8, C], BF16, tag="wq")
            nc.sync.dma_start(wq_sb[:], w_q[:, :])
            if not skip_gb:
                gb_sb = const.tile([C8, CT], F32, tag="gb")
                nc.sync.dma_start(gb_sb[:], g_b[:, :])
            if not skip_bq:
                bq_sb = const.tile([C8, 1], BF16, tag="bq")
                nc.sync.dma_start(bq_sb[:], b_q[:, :])
            if not skip_bv:
                gbv_sb = const.tile([C8, CT], F32, tag="gbv")
                nc.sync.dma_start(gbv_sb[:], gbv[:, :])
            te_f = [dram.tile([B, 512], BF16, name=f"te_f{h}") for h in range(2)]
            te_r = [dram.tile([BL, 512], BF16, name=f"te_r{h}") for h in range(2)]

            if loop_n:
                assert single, "hardware loop timing mode is single-core only"
                loop_cm = tc.For_i(0, loop_n, 1)
                loop_cm.__enter__()
            for _rep in range(repeat):
                with (
                    tc.tile_pool(name=f"xp{_rep}", bufs=12) as xp,
                    tc.tile_pool(name=f"small{_rep}", bufs=2) as sm,
                    tc.tile_pool(name=f"a4p{_rep}", bufs=3) as a4p,
                    tc.tile_pool(name=f"gr{_rep}", bufs=3) as grp,
                    tc.tile_pool(name=f"op{_rep}", bufs=3) as op,
                ):
                    xtiles = {}
                    tls = []

                    def xload(b2, eng=None):
                        # Loads the batch pair (2*b2, 2*b2+1). On the SP queue:
                        # FIFO order keeps these *behind* the g_w stream so they
                        # don't delay te-completion, then they fill the RS/prep
                        # gap and feed the epilogue. The first few go on the
                        # ACT queue (immediate) so the leading epilogue rounds
                        # aren't DMA-gated.
                        xb = xp.tile([128, 2, CT, N], BF16, tag="xb", name=f"xb{_rep}_{b2}")
                        (eng or nc.sync).dma_start(xb[:].opt(), xs[b2].opt())
                        xtiles[b2] = xb

                    # ---- Phases 1-3: G matmul in two C-halves + ReduceScatter +
                    # prep. Half A's te is reduced/transposed/prepped while half
                    # B's matmul still streams, so only half B's short tail is
                    # exposed and the PE never idles long enough to lose p-state.
                    teT_sb = const.tile([128, CT, BL], BF16, tag="teT", name=f"teT{_rep}")
                    # uT with wrapped duplicate columns so the M=32 col-packed
                    # s matmuls can slice [b : b+32] for any b
                    uT2_sb = const.tile([128, CT, 2 * BL], BF16, tag="uT2", name=f"uT2{_rep}")
                    gvr_sb = const.tile([BL, C], BF16, tag="gvr", name=f"gvr{_rep}")
                    bqd_row = const.tile([1, BL], F32, tag="bqd", name=f"bqd{_rep}") if not skip_bq else None
                    NPRE = 10  # x pairs loaded before the epilogue starts
                    te_sbh = [None, None]
                    gv_ps = [None, None]
                    kteT_ps = None

                    KB = 10  # k-tiles per DMA batch (150 = 15 * 10)
                    NPAIR = NK // 2
                    NG = NK // KB

                    with (
                        tc.tile_pool(name=f"gpsum{_rep}", bufs=2, space="PSUM") as gp,
                        tc.tile_pool(name=f"tl{_rep}", bufs=NG) as tlp,
                        tc.tile_pool(name=f"gw{_rep}", bufs=3) as gwp,
                        tc.tile_pool(name=f"tesb{_rep}", bufs=4) as tesb,
                        tc.tile_pool(name=f"pst{_rep}", bufs=2, space="PSUM") as ppst,
                        tc.tile_pool(name=f"pkte{_rep}", bufs=1, space="PSUM") as ppk,
                        tc.tile_pool(name=f"pgv{_rep}", bufs=1, space="PSUM") as ppg,
                        tc.tile_pool(name=f"pups{_rep}", bufs=1, space="PSUM") as ppu,
                        tc.tile_pool(name=f"psmall{_rep}", bufs=2) as psm,
                    ):

                        def rs_half(h, pth):
                            # evacuate the half's psums and reduce-scatter
                            for m in range(2):
                                ev = tesb.tile([128, 512], BF16, tag="tesb")
                                if m == 0:
                                    nc.vector.tensor_copy(ev[:], pth[m][:])
                                else:
                                    nc.scalar.copy(ev[:], pth[m][:])
                                nc.sync.dma_start(
                                    te_f[h][m * 128 : (m + 1) * 128, :], ev[:]
                                )
                            if single:
                                nc.sync.dma_start(te_r[h][:, :], te_f[h][0:BL, :])
                            else:
                                nc.gpsimd.collective_compute(
                                    "ReduceScatter",
                                    ALU.add,
                                    replica_groups=[list(range(N_CORES))],
                                    ins=[te_f[h].opt()],
                                    outs=[te_r[h].opt()],
                                )
                            te_sbh[h] = const.tile(
                                [BL, 512], BF16, tag="te_sb", name=f"te_sb{_rep}_{h}"
                            )
                            nc.scalar.dma_start(te_sbh[h][:], te_r[h][:, :])

                        def prep_half(h):
                            # transposes + partial kteT / gvte accumulation for
                            # the half's 4 c-tiles
                            for tt in range(4):
                                t = h * 4 + tt
                                pst = ppst.tile([128, BL], BF16, tag="pst")
                                nc.tensor.transpose(
                                    pst[:], te_sbh[h][:, tt * 128 : (tt + 1) * 128], id_sb[:]
                                )
                                if tt % 2 == 0:
                                    nc.vector.tensor_copy(teT_sb[:, t, :], pst[:])
                                else:
                                    nc.scalar.copy(teT_sb[:, t, :], pst[:])
                                if not skip_gb:
                                    nc.vector.tensor_scalar_add(
                                        teT_sb[:, t, :], teT_sb[:, t, :], gb_sb[:, t : t + 1]
                                    )
                            for tt in range(4):
                                t = h * 4 + tt
                                nc.tensor.matmul(
                                    kteT_ps[:],
                                    wkt_sb[:, t, :],
                                    teT_sb[:, t, :],
                                    start=(t == 0),
                                    stop=(t == CT - 1),
                                )
                            for h2 in range(2):
                                for tt in range(4):
                                    t = h * 4 + tt
                                    nc.tensor.matmul(
                                        gv_ps[h2][:],
                                        teT_sb[:, t, :],
                                        wvt_sb[:, t, h2 * 512 : (h2 + 1) * 512],
                                        start=(t == 0),
                                        stop=(t == CT - 1),
                                    )

                        def prep_tail():
                            # kteT/gvte evacs, uT, bqd -- after both halves landed
                            kteT_sb = psm.tile([C8, BL], BF16, tag="kteT")
                            nc.vector.tensor_copy(kteT_sb[:], kteT_ps[:])
                            for h2 in range(2):
                                if h2 == 0:
                                    nc.vector.tensor_copy(
                                        gvr_sb[:, h2 * 512 : (h2 + 1) * 512], gv_ps[h2][:]
                                    )
                                else:
                                    nc.scalar.copy(
                                        gvr_sb[:, h2 * 512 : (h2 + 1) * 512], gv_ps[h2][:]
                                    )
                            for t in range(CT):
                                u_ps = ppu.tile([128, BL], F32, tag="ups")
                                nc.tensor.matmul(
                                    u_ps[:],
                                    wq_sb[:, t * 128 : (t + 1) * 128],
                                    kteT_sb[:],
                                    start=True,
                                    stop=True,
                                )
                                if t % 2 == 0:
                                    nc.vector.tensor_copy(uT2_sb[:, t, 0:BL], u_ps[:])
                                else:
                                    nc.scalar.copy(uT2_sb[:, t, 0:BL], u_ps[:])
                            nc.vector.tensor_copy(
                                uT2_sb[:, :, BL : 2 * BL], uT2_sb[:, :, 0:BL]
                            )
                            if not skip_bq:
                                bq_ps = ppu.tile([BL, 1], F32, tag="bqps")
                                nc.tensor.matmul(bq_ps[:], kteT_sb[:], bq_sb[:], start=True, stop=True)
                                bqd_col = psm.tile([BL, 1], F32, tag="bqdc")
                                nc.vector.tensor_copy(bqd_col[:], bq_ps[:])
                                nc.sync.dma_start(bqd_row[:].rearrange("o b -> o b 1"), bqd_col[:])

                        if part in ("all", "epi"):
                            kteT_ps = ppk.tile([C8, BL], F32, tag="kte")
                            for h2 in range(2):
                                gv_ps[h2] = ppg.tile(
                                    [BL, 512], F32, tag=f"gv{h2}", name=f"gv{_rep}_{h2}"
                                )

                        def g_pass(h):
                            pth = [
                                gp.tile([128, 512], F32, tag="gp", name=f"gp{_rep}_{h}{m}")
                                for m in range(2)
                            ]
                            for g in range(NG):
                                if h == 0:
                                    tl = tlp.tile(
                                        [128, KB, B], FP8, tag="tl", name=f"tl{_rep}_{g}"
                                    )
                                    tls.append(tl)
                                    nc.sync.dma_start(
                                        tl[:].opt(), text_t[:, g * KB * B : (g + 1) * KB * B]
                                    )
                                gw_t = gwp.tile([128, KB, 512], FP8, tag="gw")
                                off = (h * NK + g * KB) * 512
                                nc.sync.dma_start(
                                    gw_t[:].opt(), g_wt[:, off : off + KB * 512]
                                )
                                for f in range(0, KB, 2):
                                    j = (g * KB + f) // 2  # pair index
                                    for m in range(2):
                                        nc.tensor.matmul(
                                            pth[m][:],
                                            tls[g][:, f : f + 2, m * 128 : (m + 1) * 128],
                                            gw_t[:, f : f + 2, :],
                                            start=(j == 0),
                                            stop=(j == NPAIR - 1),
                                            perf_mode=mybir.MatmulPerfMode.DoubleRow,
                                        )
                                # interleave half A's prep into half B's
                                # matmul stream (te_sbh[0] has landed by then)
                                if h == 1 and part == "all" and g == 8:
                                    prep_half(0)
                            return pth

                        if part in ("all", "g"):
                            for h in range(2):
                                pth = g_pass(h)
                                rs_half(h, pth)
                            if part == "all":
                                # NPRE pairs now; the rest paced from stage_a so
                                # their configs don't bury te_sb/gr on the DGE
                                for b2 in range(NPRE):
                                    xload(b2)

                        if part in ("all", "epi"):
                            if part == "epi":
                                for b2 in range(BL // 2):
                                    xload(b2)
                                for h in range(2):
                                    te_sbh[h] = const.tile(
                                        [BL, 512], BF16, tag="te_sb", name=f"te_sb{_rep}_{h}"
                                    )
                                    nc.scalar.dma_start(te_sbh[h][:], te_r[h][:, :])
                                prep_half(0)
                            prep_half(1)
                            prep_tail()

                    # ---- Phase 4: round-based epilogue, 4 batches (2 x-pairs)
                    # per round. s for a batch PAIR comes from ONE M=32 N=392
                    # matmul per c-tile: rhs = [x_b0_t | x_b1_t], lhsT = the
                    # wrapped uT2 window, so b0's s lands at row 64p cols 0:196
                    # and b1's at row 64p+1 cols 196:392 (other rows garbage
                    # but initialized). The softmax block collapses to
                    # a[n] = f(s[n]) (f = softmax-expectation of the fixed l),
                    # evaluated as a host-fitted degree-3 polynomial in 6 bf16
                    # DVE ops on the full [128,392] tile. The good a blocks are
                    # DMA-hopped into a zeroed block-diagonal tile, so each
                    # outer product covers a batch pair per c-tile via one K=2
                    # N=392 matmul. +x via DVE-TT / PE id-add / Pool (MODES).
                    NR = BL // 4  # 8 rounds
                    # per-(c-group, batch j) evac mode knobs (8 units/round)
                    MODES = [
                        ("dve", "pe", "pool", "dve"),
                        ("pe", "dve", "dve", "pool"),
                    ]
                    c3, c2, c1, c0 = fcoef
                    with (
                        tc.tile_pool(name=f"ps_s4{_rep}", bufs=2, space="PSUM") as ps_s4,
                        tc.tile_pool(name=f"ps_pr{_rep}", bufs=3, space="PSUM") as ps_pr,
                    ):
                        rs = {}

                        def stage_a(r):
                            if part == "all" and NPRE + r < BL // 2:
                                xload(NPRE + r)
                            st = {}
                            # gvte rows hop to quadrant bases (lhsT of outers)
                            gr4 = grp.tile([128, C], BF16, tag="gr4", name=f"gr4_{_rep}_{r}")
                            for j in range(4):
                                b = 4 * r + j
                                nc.scalar.dma_start(
                                    gr4[32 * j : 32 * j + 1, :], gvr_sb[b : b + 1, :]
                                )
                            # s block: col-group j accumulates batch 4r+j over
                            # the 8 c-tiles; M=32 via the wrapped uT2 columns
                            s4 = ps_s4.tile([128, 512], F32, tag="s4")
                            for j in range(4):
                                b = 4 * r + j
                                xb = xtiles[b // 2][:, b % 2]
                                for t in range(CT):
                                    nc.tensor.matmul(
                                        s4[32 * j : 32 * j + 32, 0:N],
                                        uT2_sb[:, t, b : b + 32],
                                        xb[:, t, :],
                                        start=(t == 0),
                                        stop=(t == CT - 1),
                                        tile_position=(0, 32 * j),
                                    )
                            st["s"] = s4
                            st["gr"] = gr4
                            rs[r] = st

                        def stage_b(r):
                            # a = f(s): clamp (psum read) then degree-3 poly
                            # as odd/even parts in sigma^2, all bf16 on DVE
                            st = rs[r]
                            s4 = st["s"]
                            TS = nc.vector.tensor_scalar
                            TT = nc.vector.tensor_tensor
                            sg = sm.tile([128, N], BF16, tag="sg")
                            TS(sg[:], s4[:, 0:N], FCLAMP, -FCLAMP, ALU.min, ALU.max)
                            if not skip_bq:
                                bq4 = sm.tile([128, 1], F32, tag="bq4")
                                for j in range(4):
                                    b = 4 * r + j
                                    nc.scalar.dma_start(
                                        bq4[32 * j : 32 * j + 1, :],
                                        bqd_row[0:1, b : b + 1],
                                    )
                                for j in range(4):
                                    nc.vector.tensor_scalar_add(
                                        sg[32 * j : 32 * j + 1, :],
                                        sg[32 * j : 32 * j + 1, :],
                                        bq4[32 * j : 32 * j + 1, :],
                                    )
                            s2 = sm.tile([128, N], BF16, tag="s2")
                            TT(s2[:], sg[:], sg[:], ALU.mult)
                            ao = sm.tile([128, N], BF16, tag="ao")
                            TS(ao[:], s2[:], c3, c1, ALU.mult, ALU.add)
                            be = sm.tile([128, N], BF16, tag="be")
                            TS(be[:], s2[:], c2, c0, ALU.mult, ALU.add)
                            TT(ao[:], ao[:], sg[:], ALU.mult)
                            a4 = a4p.tile([128, N], BF16, tag="a4", name=f"a4_{_rep}_{r}")
                            TT(a4[:], ao[:], be[:], ALU.add)
                            st["a"] = a4

                        def stage_c(r):
                            # outers 2-way row-packed per batch pair into wide
                            # [128,2,512] psum tiles (4 c-tiles each); +x via
                            # DVE-TT / PE id-add / Pool per MODES; DMA per pair
                            st = rs.pop(r)
                            a4 = st["a"]
                            gr4 = st["gr"]
                            obs = [
                                op.tile([128, 2, CT, N], BF16, tag="ob", name=f"ob{_rep}_{2 * r + i}")
                                for i in range(2)
                            ]
                            for p in range(2):
                                xbs = [
                                    xtiles[(4 * r + 2 * p + g) // 2][:, (4 * r + 2 * p + g) % 2]
                                    for g in range(2)
                                ]
                                for cg in range(2):
                                    prt = []
                                    for g in range(2):
                                        j = 2 * p + g
                                        mode = MODES[cg][j]
                                        pr = ps_pr.tile([128, 2, 512], F32, tag="pr")
                                        for h in range(2):
                                            for q in range(2):
                                                t = 4 * cg + 2 * h + q
                                                nc.tensor.matmul(
                                                    pr[:, h, q * N : (q + 1) * N],
                                                    gr4[32 * j : 32 * j + 1, t * 128 : (t + 1) * 128],
                                                    a4[32 * j : 32 * j + 1, :],
                                                    start=True,
                                                    stop=(mode != "pe"),
                                                    tile_position=(32 * j, 0),
                                                )
                                        prt.append((pr, mode))
                                    for g in range(2):
                                        pr, mode = prt[g]
                                        xb = xbs[g]
                                        ob = obs[p][:, g]
                                        if mode == "pe":
                                            for h in range(2):
                                                nc.tensor.matmul(
                                                    pr[:, h, 0 : 2 * N],
                                                    idf_sb[:, :],
                                                    xb[:, 4 * cg + 2 * h : 4 * cg + 2 * h + 2, :],
                                                    start=False,
                                                    stop=True,
                                                    skip_group_check=True,
                                                )
                                            nc.scalar.copy(
                                                ob[:, 4 * cg : 4 * cg + 4, :],
                                                pr[:, :, 0 : 2 * N],
                                            )
                                        elif mode == "dve":
                                            nc.vector.tensor_tensor(
                                                ob[:, 4 * cg : 4 * cg + 4, :],
                                                pr[:, :, 0 : 2 * N],
                                                xb[:, 4 * cg : 4 * cg + 4, :],
                                                ALU.add,
                                            )
                                        else:  # pool
                                            tmp = sm.tile([128, 4, N], BF16, tag="tmp")
                                            nc.scalar.copy(tmp[:], pr[:, :, 0 : 2 * N])
                                            nc.gpsimd.tensor_add(
                                                ob[:, 4 * cg : 4 * cg + 4, :],
                                                tmp[:],
                                                xb[:, 4 * cg : 4 * cg + 4, :],
                                            )
                                if not skip_bv:
                                    for g in range(2):
                                        for t in range(CT):
                                            nc.vector.tensor_scalar_add(
                                                obs[p][:, g, t, :],
                                                obs[p][:, g, t, :],
                                                gbv_sb[:, t : t + 1],
                                            )
                                nc.sync.dma_start(
                                    out[2 * r + p].opt(), obs[p][:].opt()
                                )

                        for rr in range(NR + 2 if part in ("all", "epi") else 0):
                            if 1 <= rr <= NR:
                                stage_b(rr - 1)
                            if rr < NR:
                                stage_a(rr)
                            if rr >= 2:
                                stage_c(rr - 2)
            if loop_n:
                loop_cm.__exit__(None, None, None)

    nc.compile()
    return nc


def _prep_inputs(inputs):
    """Host-side sharding. Returns in_maps for the 8 cores."""
    x = np.ascontiguousarray(inputs["x"], dtype=np.float32).reshape(B, C, N)
    text = np.ascontiguousarray(inputs["text_embed"], dtype=np.float32).reshape(B, -1)
    G_w = np.asarray(inputs["G_w"], dtype=np.float32)
    l = np.asarray(inputs["l"], dtype=np.float32).reshape(1, N)
    W_q = np.asarray(inputs["W_q"], dtype=np.float32)
    W_k = np.asarray(inputs["W_k"], dtype=np.float32)
    W_v = np.asarray(inputs["W_v"], dtype=np.float32)
    b_v = np.asarray(inputs["b_v"], dtype=np.float32)
    b_q = np.asarray(inputs["b_q"], dtype=np.float32)
    G_b = np.asarray(inputs["G_b"], dtype=np.float32)
    gamma = float(np.asarray(inputs["gamma"]).reshape(-1)[0])

    bf = ml_dtypes.bfloat16
    f8 = ml_dtypes.float8_e4m3

    def pretile(a, p=128):
        # (T*p, F) -> (p, T*F): partition-major tiling for contiguous DMA
        tp, f = a.shape
        t = tp // p
        return np.ascontiguousarray(a.reshape(t, p, f).transpose(1, 0, 2).reshape(p, t * f))

    # te is carried at GW_SCALE x through the ReduceScatter; fold the descale
    # (and gamma, for the value path) into the consumers of te.
    w_vt = pretile(np.ascontiguousarray(W_v.T * (gamma / GW_SCALE)).astype(bf))
    w_kt = pretile(np.ascontiguousarray(W_k.T / GW_SCALE).astype(bf))
    w_q = W_q.astype(bf)
    g_b_t = np.ascontiguousarray(G_b.reshape(CT, C8).T) * GW_SCALE  # (128, 8)
    gbv = np.ascontiguousarray((gamma * b_v).reshape(CT, C8).T)
    b_q_col = b_q.reshape(C8, 1).astype(bf)
    id32 = np.eye(BL, dtype=bf)
    idf = np.eye(128, dtype=bf)

    # Fit f(sigma) = sum_j l_j e^{l_j sigma} / sum_j e^{l_j sigma} (the
    # softmax-expectation of l -- a[n] = f(s[n]) pointwise) with a degree-3
    # Chebyshev polynomial over +-FCLAMP. Max fit error ~1.4e-3 of an output
    # delta that is ~1% of |x|, far inside the output tolerance.
    lv = l[0].astype(np.float64)

    def f_exact(sigma):
        z = np.outer(np.asarray(sigma, np.float64), lv)
        z -= z.max(axis=1, keepdims=True)
        e = np.exp(z)
        return (e @ lv) / e.sum(axis=1)

    gx = np.linspace(-FCLAMP, FCLAMP, 4001)
    cheb = np.polynomial.chebyshev.Chebyshev.fit(gx, f_exact(gx), 3)
    pc = cheb.convert(kind=np.polynomial.Polynomial).coef
    pc = np.pad(pc, (0, 4 - len(pc)))
    fcoef = tuple(float(c) for c in pc[::-1])  # (c3, c2, c1, c0)

    in_maps = []
    for i in range(N_CORES):
        sl = slice(i * KSH, (i + 1) * KSH)
        in_maps.append(
            {
                "text_t": pretile(np.ascontiguousarray(text[:, sl].T).astype(f8)),
                # C-halves-major so each G pass reads a contiguous half
                "g_wt": np.ascontiguousarray(
                    pretile((np.ascontiguousarray(G_w[:, sl].T) * GW_SCALE).astype(f8))
                    .reshape(128, NK, 2, 512)
                    .transpose(0, 2, 1, 3)
                    .reshape(128, NK * C)
                ),
                "xs": np.ascontiguousarray(
                    x[i * BL : (i + 1) * BL]
                    .reshape(BL // 2, 2, CT, 128, N)
                    .transpose(0, 3, 1, 2, 4)
                    .reshape(BL // 2, 128, 2 * CT * N)
                ).astype(bf),
                "w_vt": w_vt,
                "w_kt": w_kt,
                "w_q": w_q,
                "id32": id32,
                "idf": idf,
                "g_b": g_b_t,
                "b_q": b_q_col,
                "gbv": gbv,
            }
        )
    meta = {
        "gamma": gamma,
        "skip_gb": not np.any(G_b),
        "skip_bq": not np.any(b_q),
        "skip_bv": not np.any(b_v),
        "fcoef": fcoef,
    }
    return in_maps, meta


def _run(inputs, trace=False, repeat=1):
    in_maps, meta = _prep_inputs(inputs)
    nc = build(
        meta["gamma"], meta["skip_gb"], meta["skip_bq"], meta["skip_bv"],
        repeat=repeat, fcoef=meta["fcoef"],
    )
    res = run_bass_kernel_spmd(nc, in_maps, core_ids=list(range(N_CORES)), trace=trace)
    outs = [
        res.results[i]["out"]
        .astype(np.float32)
        .reshape(BL // 2, 128, 2, CT, N)
        .transpose(0, 2, 3, 1, 4)
        .reshape(BL, C, N)
        for i in range(N_CORES)
    ]
    full = np.concatenate(outs, axis=0).reshape(B, C, H, W)
    return full, res


def kernel(**inputs) -> np.ndarray:
    full, _ = _run(inputs, trace=False)
    return full


if __name__ == "__main__":
    import reference

    inputs = {k: np.asarray(v) for k, v in reference.setup_inputs().items()}
    got = kernel(**inputs)
    print("out shape:", got.shape, got.dtype)



# revision 35
# speedup vs baseline: 1.1273x; 1.0536x over previous
"""Trainium2 distributed kernel for nn_CPAM_Module (CPAM attention block).

Math collapse (verified exact vs reference, ~2.6e-8 fro rel err in f64):
  te   = text_flat @ G_w.T + G_b                      (B, C)
  te_flat = te[:, :, None] * l  (rank-1 per batch)  =>
  proj_key / proj_value are rank-1 in n; energy[b,n,m] = s[b,n]*l[m] + const(n)
  softmax over m kills the const =>
  attn[b,n,m] = softmax_m(s[b,n] * l[m])
  s[b,n] = sum_c u[b,c] x[b,c,n] + b_q.kte[b],  u = kte @ W_q, kte = te @ W_k.T
  a[b,n] = (sum_j l_j e^{l_j s}) / (sum_j e^{l_j s})
  out    = gamma * (vte[b,c] * a[b,n] + b_v[c]) + x,  vte = te @ W_v.T

Sharding: contraction (TXT=153600) split 8 ways for the big G matmul;
ReduceScatter of te (bf16) hands each core its 32 batches; epilogue is
batch-parallel. x/out traffic is B-sharded (25.7 MB each per core).

Structure:
- G matmul split into two C-halves with the text tiles resident in SBUF:
  half A's te is reduce-scattered, PE-transposed and prepped (kte/u/gvte
  partial accumulation) while half B's matmul still streams, so only half
  B's short tail is exposed and the PE never idles long enough to drop
  its p-state.
- Epilogue runs in rounds of 4 batches at partition quadrants 32j:
  s via M=32 col-packed matmuls (tile_position=(0,32j), one PSUM bank,
  extra rows garbage-but-initialized via the wrapped uT2 window).
- The whole softmax block collapses pointwise: a[n] = f(s[n]) where
  f(sigma) = sum_j l_j e^{l_j sigma} / sum_j e^{l_j sigma} is the
  softmax-expectation of the *fixed* l row -- fitted host-side with a
  degree-3 Chebyshev polynomial over +-FCLAMP (max err ~1e-3 of an output
  delta that is ~1% of |x|) and evaluated in 6 bf16 DVE ops per round,
  the first reading the s PSUM directly. This replaces the z/exp/nd/
  recip matmul-softmax pipeline entirely.
- Out outer-products 2-way row-packed (K=1 at tile_position=(32j,0))
  into 2-bank [128,2,512] PSUM tiles (4 c-tiles each); +x residual split
  DVE tensor_tensor direct / PE id-matmul add + ACT copy / ACT copy +
  Pool add per the MODES knob table (GPSIMD cannot read PSUM).
- x/out DRAM layouts carry two batches per row so each DMA moves a pair;
  NPRE pairs load after the G stream, the rest paced one per round.
- 1/GW_SCALE and gamma folded into W_k/W_v host-side.
"""

import sys

sys.path.insert(0, "/opt/trn_rl_repo")

import numpy as np
import ml_dtypes

from concourse import bass, bacc, mybir, tile
from concourse.bass_utils import run_bass_kernel_spmd

F32 = mybir.dt.float32
BF16 = mybir.dt.bfloat16
FP8 = mybir.dt.float8e4
GW_SCALE = 256.0
AF = mybir.ActivationFunctionType
ALU = mybir.AluOpType

N_CORES = 8
B, C, H, W = 256, 1024, 14, 14
N = H * W  # 196
C8 = 128
TXT = 150 * 1024
KSH = TXT // N_CORES  # 19200 txt-contraction shard per core
NK = KSH // 128  # 150 k-tiles
BL = B // N_CORES  # 32 local batches
CT = C // 128  # 8 c tiles
JT = 98  # j-tile (196 = 2*98)


FCLAMP = 120.0  # f(s) polynomial fit/clamp range


def build(gamma: float, skip_gb: bool, skip_bq: bool, skip_bv: bool, single: bool = False, repeat: int = 1, loop_n: int = 0, part: str = 'all', fcoef=(0.0,) * 4):
    # single=True builds a 1-core variant with the ReduceScatter replaced by a
    # local DMA (same bytes landing in te_rs) so TimelineSim can model it.
    nc = bacc.Bacc(
        "TRN2",
        target_bir_lowering=False,
        debug=False,
        num_devices=1 if single else N_CORES,
    )

    text_t = nc.dram_tensor("text_t", [128, NK * B], FP8, kind="ExternalInput")
    g_wt = nc.dram_tensor("g_wt", [128, NK * C], FP8, kind="ExternalInput")
    # x and out carry two batches per row so each DMA moves a pair (fewer
    # HWDGE configs + completion semaphores)
    xs = nc.dram_tensor("xs", [BL // 2, 128, 2 * CT * N], BF16, kind="ExternalInput")
    w_vt = nc.dram_tensor("w_vt", [128, CT * C], BF16, kind="ExternalInput")
    w_kt = nc.dram_tensor("w_kt", [128, CT * C8], BF16, kind="ExternalInput")
    w_q = nc.dram_tensor("w_q", [C8, C], BF16, kind="ExternalInput")
    id32 = nc.dram_tensor("id32", [BL, BL], BF16, kind="ExternalInput")
    idf = nc.dram_tensor("idf", [128, 128], BF16, kind="ExternalInput")
    g_b = nc.dram_tensor("g_b", [C8, CT], F32, kind="ExternalInput")
    b_q = nc.dram_tensor("b_q", [C8, 1], BF16, kind="ExternalInput")
    gbv = nc.dram_tensor("gbv", [C8, CT], F32, kind="ExternalInput")
    out = nc.dram_tensor("out", [BL // 2, 128, 2 * CT * N], BF16, kind="ExternalOutput")

    with tile.TileContext(nc) as tc:
        with (
            tc.tile_pool(name="const", bufs=1) as const,
            tc.tile_pool(name="dram", bufs=1, space="DRAM") as dram,
        ):
            # Constants
            id_sb = const.tile([BL, BL], BF16, tag="id32")
            nc.sync.dma_start(id_sb[:], id32[:, :])
            idf_sb = const.tile([128, 128], BF16, tag="idf")
            nc.sync.dma_start(idf_sb[:], idf[:, :])
            wvt_sb = const.tile([128, CT, C], BF16, tag="wvt")
            nc.scalar.dma_start(wvt_sb[:].opt(), w_vt[:, :])
            wkt_sb = const.tile([128, CT, C8], BF16, tag="wkt")
            nc.scalar.dma_start(wkt_sb[:].opt(), w_kt[:, :])
            wq_sb = const.tile([C8, C], BF16, tag="wq")
            nc.sync.dma_start(wq_sb[:], w_q[:, :])
            if not skip_gb:
                gb_sb = const.tile([C8, CT], F32, tag="gb")
                nc.sync.dma_start(gb_sb[:], g_b[:, :])
            if not skip_bq:
                bq_sb = const.tile([C8, 1], BF16, tag="bq")
                nc.sync.dma_start(bq_sb[:], b_q[:, :])
            if not skip_bv:
                gbv_sb = const.tile([C8, CT], F32, tag="gbv")
                nc.sync.dma_start(gbv_sb[:], gbv[:, :])
            te_f = [dram.tile([B, 512], BF16, name=f"te_f{h}") for h in range(2)]
            te_r = [dram.tile([BL, 512], BF16, name=f"te_r{h}") for h in range(2)]

            if loop_n:
                assert single, "hardware loop timing mode is single-core only"
                loop_cm = tc.For_i(0, loop_n, 1)
                loop_cm.__enter__()
            for _rep in range(repeat):
                with (
                    tc.tile_pool(name=f"xp{_rep}", bufs=14) as xp,
                    tc.tile_pool(name=f"small{_rep}", bufs=2) as sm,
                    tc.tile_pool(name=f"a4p{_rep}", bufs=3) as a4p,
                    tc.tile_pool(name=f"gr{_rep}", bufs=3) as grp,
                    tc.tile_pool(name=f"op{_rep}", bufs=3) as op,
                ):
                    xtiles = {}
                    tls = []

                    def xload(b2, eng=None):
                        # Loads the batch pair (2*b2, 2*b2+1). On the SP queue:
                        # FIFO order keeps these *behind* the g_w stream so they
                        # don't delay te-completion, then they fill the RS/prep
                        # gap and feed the epilogue. The first few go on the
                        # ACT queue (immediate) so the leading epilogue rounds
                        # aren't DMA-gated.
                        xb = xp.tile([128, 2, CT, N], BF16, tag="xb", name=f"xb{_rep}_{b2}")
                        (eng or nc.sync).dma_start(xb[:].opt(), xs[b2].opt())
                        xtiles[b2] = xb

                    # ---- Phases 1-3: G matmul in two C-halves + ReduceScatter +
                    # prep. Half A's te is reduced/transposed/prepped while half
                    # B's matmul still streams, so only half B's short tail is
                    # exposed and the PE never idles long enough to lose p-state.
                    teT_sb = const.tile([128, CT, BL], BF16, tag="teT", name=f"teT{_rep}")
                    # uT with wrapped duplicate columns so the M=32 col-packed
                    # s matmuls can slice [b : b+32] for any b
                    uT2_sb = const.tile([128, CT, 2 * BL], BF16, tag="uT2", name=f"uT2{_rep}")
                    gvr_sb = const.tile([BL, C], BF16, tag="gvr", name=f"gvr{_rep}")
                    bqd_row = const.tile([1, BL], F32, tag="bqd", name=f"bqd{_rep}") if not skip_bq else None
                    NPRE = 10  # x pairs loaded before the epilogue starts
                    te_sbh = [None, None]
                    gv_ps = [None, None]
                    kteT_ps = None

                    KB = 10  # k-tiles per DMA batch (150 = 15 * 10)
                    NPAIR = NK // 2
                    NG = NK // KB

                    with (
                        tc.tile_pool(name=f"gpsum{_rep}", bufs=2, space="PSUM") as gp,
                        tc.tile_pool(name=f"tl{_rep}", bufs=NG) as tlp,
                        tc.tile_pool(name=f"gw{_rep}", bufs=4) as gwp,
                        tc.tile_pool(name=f"tesb{_rep}", bufs=4) as tesb,
                        tc.tile_pool(name=f"pst{_rep}", bufs=2, space="PSUM") as ppst,
                        tc.tile_pool(name=f"pkte{_rep}", bufs=1, space="PSUM") as ppk,
                        tc.tile_pool(name=f"pgv{_rep}", bufs=1, space="PSUM") as ppg,
                        tc.tile_pool(name=f"pups{_rep}", bufs=1, space="PSUM") as ppu,
                        tc.tile_pool(name=f"psmall{_rep}", bufs=2) as psm,
                    ):

                        def rs_half(h, pth):
                            # evacuate the half's psums and reduce-scatter
                            for m in range(2):
                                ev = tesb.tile([128, 512], BF16, tag="tesb")
                                if m == 0:
                                    nc.vector.tensor_copy(ev[:], pth[m][:])
                                else:
                                    nc.scalar.copy(ev[:], pth[m][:])
                                nc.sync.dma_start(
                                    te_f[h][m * 128 : (m + 1) * 128, :], ev[:]
                                )
                            if single:
                                nc.sync.dma_start(te_r[h][:, :], te_f[h][0:BL, :])
                            else:
                                nc.gpsimd.collective_compute(
                                    "ReduceScatter",
                                    ALU.add,
                                    replica_groups=[list(range(N_CORES))],
                                    ins=[te_f[h].opt()],
                                    outs=[te_r[h].opt()],
                                )
                            te_sbh[h] = const.tile(
                                [BL, 512], BF16, tag="te_sb", name=f"te_sb{_rep}_{h}"
                            )
                            nc.scalar.dma_start(te_sbh[h][:], te_r[h][:, :])

                        def prep_half(h):
                            # transposes + partial kteT / gvte accumulation for
                            # the half's 4 c-tiles
                            for tt in range(4):
                                t = h * 4 + tt
                                pst = ppst.tile([128, BL], BF16, tag="pst")
                                nc.tensor.transpose(
                                    pst[:], te_sbh[h][:, tt * 128 : (tt + 1) * 128], id_sb[:]
                                )
                                if tt % 2 == 0:
                                    nc.vector.tensor_copy(teT_sb[:, t, :], pst[:])
                                else:
                                    nc.scalar.copy(teT_sb[:, t, :], pst[:])
                                if not skip_gb:
                                    nc.vector.tensor_scalar_add(
                                        teT_sb[:, t, :], teT_sb[:, t, :], gb_sb[:, t : t + 1]
                                    )
                            for tt in range(4):
                                t = h * 4 + tt
                                nc.tensor.matmul(
                                    kteT_ps[:],
                                    wkt_sb[:, t, :],
                                    teT_sb[:, t, :],
                                    start=(t == 0),
                                    stop=(t == CT - 1),
                                )
                            for h2 in range(2):
                                for tt in range(4):
                                    t = h * 4 + tt
                                    nc.tensor.matmul(
                                        gv_ps[h2][:],
                                        teT_sb[:, t, :],
                                        wvt_sb[:, t, h2 * 512 : (h2 + 1) * 512],
                                        start=(t == 0),
                                        stop=(t == CT - 1),
                                    )

                        def prep_tail():
                            # kteT/gvte evacs, uT, bqd -- after both halves landed
                            kteT_sb = psm.tile([C8, BL], BF16, tag="kteT")
                            nc.vector.tensor_copy(kteT_sb[:], kteT_ps[:])
                            for h2 in range(2):
                                if h2 == 0:
                                    nc.vector.tensor_copy(
                                        gvr_sb[:, h2 * 512 : (h2 + 1) * 512], gv_ps[h2][:]
                                    )
                                else:
                                    nc.scalar.copy(
                                        gvr_sb[:, h2 * 512 : (h2 + 1) * 512], gv_ps[h2][:]
                                    )
                            for t in range(CT):
                                u_ps = ppu.tile([128, BL], F32, tag="ups")
                                nc.tensor.matmul(
                                    u_ps[:],
                                    wq_sb[:, t * 128 : (t + 1) * 128],
                                    kteT_sb[:],
                                    start=True,
                                    stop=True,
                                )
                                if t % 2 == 0:
                                    nc.vector.tensor_copy(uT2_sb[:, t, 0:BL], u_ps[:])
                                else:
                                    nc.scalar.copy(uT2_sb[:, t, 0:BL], u_ps[:])
                            nc.vector.tensor_copy(
                                uT2_sb[:, :, BL : 2 * BL], uT2_sb[:, :, 0:BL]
                            )
                            if not skip_bq:
                                bq_ps = ppu.tile([BL, 1], F32, tag="bqps")
                                nc.tensor.matmul(bq_ps[:], kteT_sb[:], bq_sb[:], start=True, stop=True)
                                bqd_col = psm.tile([BL, 1], F32, tag="bqdc")
                                nc.vector.tensor_copy(bqd_col[:], bq_ps[:])
                                nc.sync.dma_start(bqd_row[:].rearrange("o b -> o b 1"), bqd_col[:])

                        if part in ("all", "epi"):
                            kteT_ps = ppk.tile([C8, BL], F32, tag="kte")
                            for h2 in range(2):
                                gv_ps[h2] = ppg.tile(
                                    [BL, 512], F32, tag=f"gv{h2}", name=f"gv{_rep}_{h2}"
                                )

                        def g_pass(h):
                            pth = [
                                gp.tile([128, 512], F32, tag="gp", name=f"gp{_rep}_{h}{m}")
                                for m in range(2)
                            ]
                            for g in range(NG):
                                if h == 0:
                                    tl = tlp.tile(
                                        [128, KB, B], FP8, tag="tl", name=f"tl{_rep}_{g}"
                                    )
                                    tls.append(tl)
                                    nc.sync.dma_start(
                                        tl[:].opt(), text_t[:, g * KB * B : (g + 1) * KB * B]
                                    )
                                gw_t = gwp.tile([128, KB, 512], FP8, tag="gw")
                                off = (h * NK + g * KB) * 512
                                nc.sync.dma_start(
                                    gw_t[:].opt(), g_wt[:, off : off + KB * 512]
                                )
                                for f in range(0, KB, 2):
                                    j = (g * KB + f) // 2  # pair index
                                    for m in range(2):
                                        nc.tensor.matmul(
                                            pth[m][:],
                                            tls[g][:, f : f + 2, m * 128 : (m + 1) * 128],
                                            gw_t[:, f : f + 2, :],
                                            start=(j == 0),
                                            stop=(j == NPAIR - 1),
                                            perf_mode=mybir.MatmulPerfMode.DoubleRow,
                                        )
                                # interleave half A's prep into half B's
                                # matmul stream (te_sbh[0] has landed by then)
                                if h == 1 and part == "all" and g == 8:
                                    prep_half(0)
                            return pth

                        if part in ("all", "g"):
                            for h in range(2):
                                pth = g_pass(h)
                                rs_half(h, pth)
                            if part == "all":
                                # NPRE pairs now; the rest paced from stage_a so
                                # their configs don't bury te_sb/gr on the DGE
                                for b2 in range(NPRE):
                                    xload(b2)

                        if part in ("all", "epi"):
                            if part == "epi":
                                for b2 in range(BL // 2):
                                    xload(b2)
                                for h in range(2):
                                    te_sbh[h] = const.tile(
                                        [BL, 512], BF16, tag="te_sb", name=f"te_sb{_rep}_{h}"
                                    )
                                    nc.scalar.dma_start(te_sbh[h][:], te_r[h][:, :])
                                prep_half(0)
                            prep_half(1)
                            prep_tail()

                    # ---- Phase 4: round-based epilogue, 4 batches (2 x-pairs)
                    # per round. s for a batch PAIR comes from ONE M=32 N=392
                    # matmul per c-tile: rhs = [x_b0_t | x_b1_t], lhsT = the
                    # wrapped uT2 window, so b0's s lands at row 64p cols 0:196
                    # and b1's at row 64p+1 cols 196:392 (other rows garbage
                    # but initialized). The softmax block collapses to
                    # a[n] = f(s[n]) (f = softmax-expectation of the fixed l),
                    # evaluated as a host-fitted degree-3 polynomial in 6 bf16
                    # DVE ops on the full [128,392] tile. The good a blocks are
                    # DMA-hopped into a zeroed block-diagonal tile, so each
                    # outer product covers a batch pair per c-tile via one K=2
                    # N=392 matmul. +x via DVE-TT / PE id-add / Pool (MODES).
                    NR = BL // 4  # 8 rounds
                    # per-(c-group, batch j) evac mode knobs (8 units/round)
                    MODES = [
                        ("dve", "pe", "pool", "dve"),
                        ("pe", "dve", "pe", "pool"),
                    ]
                    c3, c2, c1, c0 = fcoef
                    with (
                        tc.tile_pool(name=f"ps_s4{_rep}", bufs=2, space="PSUM") as ps_s4,
                        tc.tile_pool(name=f"ps_pr{_rep}", bufs=3, space="PSUM") as ps_pr,
                    ):
                        rs = {}

                        def stage_a(r):
                            if part == "all" and NPRE + r < BL // 2:
                                xload(NPRE + r)
                            st = {}
                            # gvte rows hop to quadrant bases (lhsT of outers)
                            gr4 = grp.tile([128, C], BF16, tag="gr4", name=f"gr4_{_rep}_{r}")
                            for j in range(4):
                                b = 4 * r + j
                                nc.scalar.dma_start(
                                    gr4[32 * j : 32 * j + 1, :], gvr_sb[b : b + 1, :]
                                )
                            # s block: col-group j accumulates batch 4r+j over
                            # the 8 c-tiles; M=32 via the wrapped uT2 columns
                            s4 = ps_s4.tile([128, 512], F32, tag="s4")
                            for j in range(4):
                                b = 4 * r + j
                                xb = xtiles[b // 2][:, b % 2]
                                for t in range(CT):
                                    nc.tensor.matmul(
                                        s4[32 * j : 32 * j + 32, 0:N],
                                        uT2_sb[:, t, b : b + 32],
                                        xb[:, t, :],
                                        start=(t == 0),
                                        stop=(t == CT - 1),
                                        tile_position=(0, 32 * j),
                                    )
                            st["s"] = s4
                            st["gr"] = gr4
                            rs[r] = st

                        def stage_b(r):
                            # a = f(s): clamp (psum read) then degree-3 poly
                            # as odd/even parts in sigma^2, all bf16 on DVE
                            st = rs[r]
                            s4 = st["s"]
                            TS = nc.vector.tensor_scalar
                            TT = nc.vector.tensor_tensor
                            sg = sm.tile([128, N], BF16, tag="sg")
                            TS(sg[:], s4[:, 0:N], FCLAMP, -FCLAMP, ALU.min, ALU.max)
                            if not skip_bq:
                                bq4 = sm.tile([128, 1], F32, tag="bq4")
                                for j in range(4):
                                    b = 4 * r + j
                                    nc.scalar.dma_start(
                                        bq4[32 * j : 32 * j + 1, :],
                                        bqd_row[0:1, b : b + 1],
                                    )
                                for j in range(4):
                                    nc.vector.tensor_scalar_add(
                                        sg[32 * j : 32 * j + 1, :],
                                        sg[32 * j : 32 * j + 1, :],
                                        bq4[32 * j : 32 * j + 1, :],
                                    )
                            s2 = sm.tile([128, N], BF16, tag="s2")
                            TT(s2[:], sg[:], sg[:], ALU.mult)
                            ao = sm.tile([128, N], BF16, tag="ao")
                            TS(ao[:], s2[:], c3, c1, ALU.mult, ALU.add)
                            be = sm.tile([128, N], BF16, tag="be")
                            TS(be[:], s2[:], c2, c0, ALU.mult, ALU.add)
                            TT(ao[:], ao[:], sg[:], ALU.mult)
                            a4 = a4p.tile([128, N], BF16, tag="a4", name=f"a4_{_rep}_{r}")
                            TT(a4[:], ao[:], be[:], ALU.add)
                            st["a"] = a4

                        def stage_c(r):
                            # outers 2-way row-packed per batch pair into wide
                            # [128,2,512] psum tiles (4 c-tiles each); +x via
                            # DVE-TT / PE id-add / Pool per MODES; DMA per pair
                            st = rs.pop(r)
                            a4 = st["a"]
                            gr4 = st["gr"]
                            obs = [
                                op.tile([128, 2, CT, N], BF16, tag="ob", name=f"ob{_rep}_{2 * r + i}")
                                for i in range(2)
                            ]
                            for p in range(2):
                                xbs = [
                                    xtiles[(4 * r + 2 * p + g) // 2][:, (4 * r + 2 * p + g) % 2]
                                    for g in range(2)
                                ]
                                for cg in range(2):
                                    prt = []
                                    for g in range(2):
                                        j = 2 * p + g
                                        mode = MODES[cg][j]
                                        pr = ps_pr.tile([128, 2, 512], F32, tag="pr")
                                        for h in range(2):
                                            for q in range(2):
                                                t = 4 * cg + 2 * h + q
                                                nc.tensor.matmul(
                                                    pr[:, h, q * N : (q + 1) * N],
                                                    gr4[32 * j : 32 * j + 1, t * 128 : (t + 1) * 128],
                                                    a4[32 * j : 32 * j + 1, :],
                                                    start=True,
                                                    stop=(mode != "pe"),
                                                    tile_position=(32 * j, 0),
                                                )
                                        prt.append((pr, mode))
                                    for g in range(2):
                                        pr, mode = prt[g]
                                        xb = xbs[g]
                                        ob = obs[p][:, g]
                                        if mode == "pe":
                                            for h in range(2):
                                                nc.tensor.matmul(
                                                    pr[:, h, 0 : 2 * N],
                                                    idf_sb[:, :],
                                                    xb[:, 4 * cg + 2 * h : 4 * cg + 2 * h + 2, :],
                                                    start=False,
                                                    stop=True,
                                                    skip_group_check=True,
                                                )
                                            nc.scalar.copy(
                                                ob[:, 4 * cg : 4 * cg + 4, :],
                                                pr[:, :, 0 : 2 * N],
                                            )
                                        elif mode == "dve":
                                            nc.vector.tensor_tensor(
                                                ob[:, 4 * cg : 4 * cg + 4, :],
                                                pr[:, :, 0 : 2 * N],
                                                xb[:, 4 * cg : 4 * cg + 4, :],
                                                ALU.add,
                                            )
                                        else:  # pool
                                            tmp = sm.tile([128, 4, N], BF16, tag="tmp")
                                            nc.scalar.copy(tmp[:], pr[:, :, 0 : 2 * N])
                                            nc.gpsimd.tensor_add(
                                                ob[:, 4 * cg : 4 * cg + 4, :],
                                                tmp[:],
                                                xb[:, 4 * cg : 4 * cg + 4, :],
                                            )
                                if not skip_bv:
                                    for g in range(2):
                                        for t in range(CT):
                                            nc.vector.tensor_scalar_add(
                                                obs[p][:, g, t, :],
                                                obs[p][:, g, t, :],
                                                gbv_sb[:, t : t + 1],
                                            )
                                nc.sync.dma_start(
                                    out[2 * r + p].opt(), obs[p][:].opt()
                                )

                        for rr in range(NR + 2 if part in ("all", "epi") else 0):
                            if 1 <= rr <= NR:
                                stage_b(rr - 1)
                            if rr < NR:
                                stage_a(rr)
                            if rr >= 2:
                                stage_c(rr - 2)
            if loop_n:
                loop_cm.__exit__(None, None, None)

    nc.compile()
    return nc


def _prep_inputs(inputs):
    """Host-side sharding. Returns in_maps for the 8 cores."""
    x = np.ascontiguousarray(inputs["x"], dtype=np.float32).reshape(B, C, N)
    text = np.ascontiguousarray(inputs["text_embed"], dtype=np.float32).reshape(B, -1)
    G_w = np.asarray(inputs["G_w"], dtype=np.float32)
    l = np.asarray(inputs["l"], dtype=np.float32).reshape(1, N)
    W_q = np.asarray(inputs["W_q"], dtype=np.float32)
    W_k = np.asarray(inputs["W_k"], dtype=np.float32)
    W_v = np.asarray(inputs["W_v"], dtype=np.float32)
    b_v = np.asarray(inputs["b_v"], dtype=np.float32)
    b_q = np.asarray(inputs["b_q"], dtype=np.float32)
    G_b = np.asarray(inputs["G_b"], dtype=np.float32)
    gamma = float(np.asarray(inputs["gamma"]).reshape(-1)[0])

    bf = ml_dtypes.bfloat16
    f8 = ml_dtypes.float8_e4m3

    def pretile(a, p=128):
        # (T*p, F) -> (p, T*F): partition-major tiling for contiguous DMA
        tp, f = a.shape
        t = tp // p
        return np.ascontiguousarray(a.reshape(t, p, f).transpose(1, 0, 2).reshape(p, t * f))

    # te is carried at GW_SCALE x through the ReduceScatter; fold the descale
    # (and gamma, for the value path) into the consumers of te.
    w_vt = pretile(np.ascontiguousarray(W_v.T * (gamma / GW_SCALE)).astype(bf))
    w_kt = pretile(np.ascontiguousarray(W_k.T / GW_SCALE).astype(bf))
    w_q = W_q.astype(bf)
    g_b_t = np.ascontiguousarray(G_b.reshape(CT, C8).T) * GW_SCALE  # (128, 8)
    gbv = np.ascontiguousarray((gamma * b_v).reshape(CT, C8).T)
    b_q_col = b_q.reshape(C8, 1).astype(bf)
    id32 = np.eye(BL, dtype=bf)
    idf = np.eye(128, dtype=bf)

    # Fit f(sigma) = sum_j l_j e^{l_j sigma} / sum_j e^{l_j sigma} (the
    # softmax-expectation of l -- a[n] = f(s[n]) pointwise) with a degree-3
    # Chebyshev polynomial over +-FCLAMP. Max fit error ~1.4e-3 of an output
    # delta that is ~1% of |x|, far inside the output tolerance.
    lv = l[0].astype(np.float64)

    def f_exact(sigma):
        z = np.outer(np.asarray(sigma, np.float64), lv)
        z -= z.max(axis=1, keepdims=True)
        e = np.exp(z)
        return (e @ lv) / e.sum(axis=1)

    gx = np.linspace(-FCLAMP, FCLAMP, 4001)
    cheb = np.polynomial.chebyshev.Chebyshev.fit(gx, f_exact(gx), 3)
    pc = cheb.convert(kind=np.polynomial.Polynomial).coef
    pc = np.pad(pc, (0, 4 - len(pc)))
    fcoef = tuple(float(c) for c in pc[::-1])  # (c3, c2, c1, c0)

    in_maps = []
    for i in range(N_CORES):
        sl = slice(i * KSH, (i + 1) * KSH)
        in_maps.append(
            {
                "text_t": pretile(np.ascontiguousarray(text[:, sl].T).astype(f8)),
                # C-halves-major so each G pass reads a contiguous half
                "g_wt": np.ascontiguousarray(
                    pretile((np.ascontiguousarray(G_w[:, sl].T) * GW_SCALE).astype(f8))
                    .reshape(128, NK, 2, 512)
                    .transpose(0, 2, 1, 3)
                    .reshape(128, NK * C)
                ),
                "xs": np.ascontiguousarray(
                    x[i * BL : (i + 1) * BL]
                    .reshape(BL // 2, 2, CT, 128, N)
                    .transpose(0, 3, 1, 2, 4)
                    .reshape(BL // 2, 128, 2 * CT * N)
                ).astype(bf),
                "w_vt": w_vt,
                "w_kt": w_kt,
                "w_q": w_q,
                "id32": id32,
                "idf": idf,
                "g_b": g_b_t,
                "b_q": b_q_col,
                "gbv": gbv,
            }
        )
    meta = {
        "gamma": gamma,
        "skip_gb": not np.any(G_b),
        "skip_bq": not np.any(b_q),
        "skip_bv": not np.any(b_v),
        "fcoef": fcoef,
    }
    return in_maps, meta


def _run(inputs, trace=False, repeat=1):
    in_maps, meta = _prep_inputs(inputs)
    nc = build(
        meta["gamma"], meta["skip_gb"], meta["skip_bq"], meta["skip_bv"],
        repeat=repeat, fcoef=meta["fcoef"],
    )
    res = run_bass_kernel_spmd(nc, in_maps, core_ids=list(range(N_CORES)), trace=trace)
    outs = [
        res.results[i]["out"]
        .astype(np.float32)
        .reshape(BL // 2, 128, 2, CT, N)
        .transpose(0, 2, 3, 1, 4)
        .reshape(BL, C, N)
        for i in range(N_CORES)
    ]
    full = np.concatenate(outs, axis=0).reshape(B, C, H, W)
    return full, res


def kernel(**inputs) -> np.ndarray:
    full, _ = _run(inputs, trace=False)
    return full


if __name__ == "__main__":
    import reference

    inputs = {k: np.asarray(v) for k, v in reference.setup_inputs().items()}
    got = kernel(**inputs)
    print("out shape:", got.shape, got.dtype)

